# revision 8
# baseline (speedup 1.0000x reference)
"""Bahdanau attention with coverage — TRN2 Bass kernel, data-parallel over batch.

Full-size problem: B=64, S=2048, ENC=1024, DEC=512, ATTN=1024, 8 NeuronCores.
Each core handles B_loc = 8 batches. All matmuls run in float32r (full PE rate,
~1.5e-4 rel err). Per batch:
  1. DMA X = encoder_outputs[b] ([S, ENC] fp32), PE-transpose into XT [ENC, S]
     (f32r, SBUF-resident).
  2. combined^T [A, S] tiles = WencT.T @ XT (+ rank-1 coverage row folded in as
     a K=1 matmul into the same PSUM accumulation group).
  3. tanh via ACT with per-partition bias = dec_feat^T (precomputed on PE).
  4. scores = v^T @ tanh(...) on PE (M=1 matmuls).
  5. masked softmax on DVE/ACT (fp32).
  6. context via DVE tensor_tensor_reduce against XT (no second X read).
"""
import sys

sys.path.insert(0, "/opt/trn_rl_repo")

import numpy as np
import concourse.bass as bass
import concourse.tile as tile
from concourse import bacc, mybir
from concourse.bass_utils import run_bass_kernel_spmd
from concourse.masks import make_identity

F32 = mybir.dt.float32
F32R = mybir.dt.float32r
I32 = mybir.dt.int32
AF = mybir.ActivationFunctionType
OP = mybir.AluOpType

N_CORES = 8
B, S, ENC, DEC, ATT = 64, 2048, 1024, 512, 1024
B_LOC = B // N_CORES


def build_nc(b_loc=B_LOC, s=S, e_dim=ENC, d_dim=DEC, a_dim=ATT, sc=512,
             tanh_bufs=9, xnat_bufs=3, psm_bufs=3, stages=6):
    """Build the per-core Bass program (SPMD: same program, different data)."""
    nc = bacc.Bacc(trn_type="TRN2")
    NE, NA, ND, NSB = e_dim // 128, a_dim // 128, d_dim // 128, s // 128
    NSC = s // sc
    SBPC = sc // 128  # s-blocks per chunk

    dh = nc.declare_dram_parameter("decoder_hidden", [b_loc, d_dim], F32, isOutput=False)
    enc = nc.declare_dram_parameter("encoder_outputs", [b_loc, s, e_dim], F32, isOutput=False)
    msk = nc.declare_dram_parameter("encoder_mask", [b_loc, s], I32, isOutput=False)
    cov = nc.declare_dram_parameter("coverage", [b_loc, s], F32, isOutput=False)
    wenc = nc.declare_dram_parameter("W_encoder", [a_dim, e_dim], F32, isOutput=False)
    wdec = nc.declare_dram_parameter("W_decoder", [a_dim, d_dim], F32, isOutput=False)
    wcov = nc.declare_dram_parameter("W_coverage", [a_dim, 1], F32, isOutput=False)
    vv = nc.declare_dram_parameter("v", [1, a_dim], F32, isOutput=False)
    ctx_out = nc.declare_dram_parameter("context", [b_loc, e_dim], F32, isOutput=True)
    attn_out = nc.declare_dram_parameter("attn", [b_loc, s], F32, isOutput=True)

    with tile.TileContext(nc) as tc:
        with tc.tile_pool(name="persist", bufs=1) as persist:
            ident = persist.tile([128, 128], F32)
            make_identity(nc, ident)

            wencT = persist.tile([128, NE, a_dim], F32R)    # [e_part, e_tile, a]
            xt = persist.tile([128, NE, s], F32R)           # [e_part, e_tile, s]
            decT = persist.tile([128, NA, b_loc], F32)      # [a_part, a_tile, b]
            wcovT = persist.tile([1, a_dim], F32R)
            vT = persist.tile([128, NA], F32R)

            # ---------- precompute phase (scoped pools; freed afterwards) ----
            with tc.tile_pool(name="pre", bufs=2) as pre, \
                 tc.tile_pool(name="pre1", bufs=1) as pre1, \
                 tc.tile_pool(name="ps_pre", bufs=3, space="PSUM") as ps_pre:
                # WencT: load natural [a_part, e], transpose 128x128 blocks
                for a_t in range(NA):
                    wn = pre.tile([128, e_dim], F32, tag="wenc_nat")
                    nc.sync.dma_start(out=wn, in_=wenc[a_t * 128:(a_t + 1) * 128, :])
                    for eg in range((NE + 3) // 4):
                        n_sub = min(4, NE - eg * 4)
                        pt = ps_pre.tile([128, 512], F32, tag="pp")
                        for j in range(n_sub):
                            nc.tensor.transpose(
                                pt[:, j * 128:(j + 1) * 128],
                                wn[:, (eg * 4 + j) * 128:(eg * 4 + j + 1) * 128],
                                ident)
                        nc.vector.tensor_copy(
                            out=wencT[:, eg * 4:eg * 4 + n_sub,
                                      a_t * 128:(a_t + 1) * 128],
                            in_=pt[:, :n_sub * 128].rearrange(
                                "p (j q) -> p j q", j=n_sub))

                # dhT: [b_loc, d] -> d-tiles of [128, b_loc]
                dh_sb = pre1.tile([b_loc, d_dim], F32)
                nc.sync.dma_start(out=dh_sb, in_=dh[:])
                dhT = pre1.tile([128, ND, b_loc], F32R)
                for j in range(ND):
                    pt = ps_pre.tile([128, b_loc], F32, tag="pp")
                    nc.tensor.transpose(pt, dh_sb[:, j * 128:(j + 1) * 128],
                                        ident[:b_loc, :b_loc])
                    nc.vector.tensor_copy(out=dhT[:, j], in_=pt)

                # WdecT: [a, d] -> d-tiles of [128, a]
                wdecT = pre1.tile([128, ND, a_dim], F32R)
                for a_t in range(NA):
                    wn = pre.tile([128, d_dim], F32, tag="wdec_nat")
                    nc.sync.dma_start(out=wn, in_=wdec[a_t * 128:(a_t + 1) * 128, :])
                    for j in range(ND):
                        pt = ps_pre.tile([128, 128], F32, tag="pp")
                        nc.tensor.transpose(pt, wn[:, j * 128:(j + 1) * 128], ident)
                        nc.vector.tensor_copy(
                            out=wdecT[:, j, a_t * 128:(a_t + 1) * 128], in_=pt)

                # decT[a_part, a_tile, b] = Wdec @ dh^T  (dec_feat transposed)
                for a_t in range(NA):
                    pd = ps_pre.tile([128, b_loc], F32, tag="pp")
                    for j in range(ND):
                        nc.tensor.matmul(pd, wdecT[:, j, a_t * 128:(a_t + 1) * 128],
                                         dhT[:, j], start=(j == 0), stop=(j == ND - 1))
                    nc.vector.tensor_copy(out=decT[:, a_t], in_=pd)

                # WcovT row [1, a]
                wcn = pre1.tile([128, NA], F32)
                nc.sync.dma_start(out=wcn, in_=wcov.rearrange("(t p) o -> p (t o)", p=128))
                for a_t in range(NA):
                    pt = ps_pre.tile([1, 128], F32, tag="pp")
                    nc.tensor.transpose(pt, wcn[:, a_t:a_t + 1], ident)
                    nc.vector.tensor_copy(out=wcovT[0:1, a_t * 128:(a_t + 1) * 128],
                                          in_=pt)

                # vT [128, NA]
                v_sb = pre1.tile([1, a_dim], F32)
                nc.sync.dma_start(out=v_sb, in_=vv[:])
                for a_t in range(NA):
                    pt = ps_pre.tile([128, 1], F32, tag="pp")
                    nc.tensor.transpose(pt, v_sb[0:1, a_t * 128:(a_t + 1) * 128],
                                        ident[:1, :1])
                    nc.vector.tensor_copy(out=vT[:, a_t:a_t + 1], in_=pt)

            # ---------- main per-batch pipeline --------------------------------
            with tc.tile_pool(name="xnat", bufs=xnat_bufs) as xpool, \
                 tc.tile_pool(name="tanhp", bufs=tanh_bufs) as tanh_pool, \
                 tc.tile_pool(name="covp", bufs=1) as covpool, \
                 tc.tile_pool(name="rowp", bufs=1) as rowpool, \
                 tc.tile_pool(name="bcp", bufs=2) as bcpool, \
                 tc.tile_pool(name="scrp", bufs=2) as scrpool, \
                 tc.tile_pool(name="smallp", bufs=2) as small, \
                 tc.tile_pool(name="ctxp", bufs=2) as ctxpool, \
                 tc.tile_pool(name="dramp", bufs=2, space="DRAM") as dram_pool, \
                 tc.tile_pool(name="ps_t", bufs=2, space="PSUM") as ps_t, \
                 tc.tile_pool(name="ps_m", bufs=psm_bufs, space="PSUM") as ps_m, \
                 tc.tile_pool(name="ps_s", bufs=2, space="PSUM") as ps_s:
                if stages < 6:
                    zrow = rowpool.tile([1, max(s, e_dim)], F32, tag="zrow")
                    nc.vector.memset(zrow, 0.0)
                    for b in range(b_loc):
                        nc.sync.dma_start(out=attn_out[b:b + 1, :], in_=zrow[0:1, :s])
                        nc.sync.dma_start(out=ctx_out[b:b + 1, :], in_=zrow[0:1, :e_dim])
                for b in range(b_loc):
                    if stages < 2:
                        break
                    # coverage row, rounded to f32r
                    cov_raw = covpool.tile([1, s], F32, tag="cov_raw")
                    nc.sync.dma_start(out=cov_raw, in_=cov[b:b + 1, :])
                    cov_r = covpool.tile([1, s], F32R, tag="cov_r")
                    nc.vector.tensor_copy(out=cov_r, in_=cov_raw)

                    # X load + PE transpose into xt
                    for sb in range(NSB):
                        xn = xpool.tile([128, e_dim], F32, tag="xnat")
                        nc.sync.dma_start(out=xn, in_=enc[b, sb * 128:(sb + 1) * 128, :])
                        for eg in range((NE + 3) // 4):
                            n_sub = min(4, NE - eg * 4)
                            pt = ps_t.tile([128, 512], F32, tag="ps_t")
                            for j in range(n_sub):
                                nc.tensor.transpose(
                                    pt[:, j * 128:(j + 1) * 128],
                                    xn[:, (eg * 4 + j) * 128:(eg * 4 + j + 1) * 128],
                                    ident)
                            nc.vector.tensor_copy(
                                out=xt[:, eg * 4:eg * 4 + n_sub,
                                       sb * 128:(sb + 1) * 128],
                                in_=pt[:, :n_sub * 128].rearrange(
                                    "p (j q) -> p j q", j=n_sub))

                    scores_row = rowpool.tile([1, s], F32, tag="scores_row")
                    work_row = rowpool.tile([1, s], F32, tag="work_row")
                    p_row = rowpool.tile([1, s], F32, tag="p_row")

                    if stages < 3:
                        continue
                    # main matmuls + tanh + score matmuls, per s-chunk
                    for scix in range(NSC):
                        ssl = slice(scix * sc, (scix + 1) * sc)
                        tanh_tiles = []
                        for a_t in range(NA):
                            pm = ps_m.tile([128, sc], F32, tag="ps_m")
                            for e_t in range(NE):
                                nc.tensor.matmul(
                                    pm, wencT[:, e_t, a_t * 128:(a_t + 1) * 128],
                                    xt[:, e_t, ssl], start=(e_t == 0), stop=False)
                            nc.tensor.matmul(
                                pm, wcovT[0:1, a_t * 128:(a_t + 1) * 128],
                                cov_r[0:1, ssl], start=False, stop=True)
                            th = tanh_pool.tile([128, sc], F32R, tag="tanh")
                            nc.scalar.activation(out=th, in_=pm, func=AF.Tanh,
                                                 bias=decT[:, a_t, b:b + 1], scale=1.0)
                            tanh_tiles.append(th)
                        if stages < 4:
                            continue
                        pss = ps_s.tile([1, sc], F32, tag="ps_s")
                        for a_t in range(NA):
                            nc.tensor.matmul(pss, vT[:, a_t:a_t + 1], tanh_tiles[a_t],
                                             start=(a_t == 0), stop=(a_t == NA - 1))
                        nc.vector.tensor_copy(out=scores_row[0:1, ssl], in_=pss)

                    if stages < 5:
                        continue
                    # masked softmax for row b (fp32, partition-0 row tiles)
                    nc.sync.dma_start(out=p_row.bitcast(I32), in_=msk[b:b + 1, :])
                    nc.vector.tensor_copy(out=work_row, in_=p_row.bitcast(I32))
                    nc.vector.scalar_tensor_tensor(
                        out=p_row, in0=scores_row, scalar=30000.0,
                        in1=work_row, op0=OP.add, op1=OP.mult)
                    nc.vector.tensor_scalar(
                        out=work_row, in0=p_row,
                        scalar1=-30000.0, scalar2=None, op0=OP.add)
                    rmax = small.tile([1, 1], F32, tag="rmax")
                    nc.vector.reduce_max(out=rmax, in_=work_row,
                                         axis=mybir.AxisListType.X)
                    nmax = small.tile([1, 1], F32, tag="nmax")
                    nc.vector.tensor_scalar_mul(nmax, rmax, -1.0)
                    ssum = small.tile([1, 1], F32, tag="ssum")
                    nc.scalar.activation(out=p_row, in_=work_row,
                                         func=AF.Exp, bias=nmax, scale=1.0,
                                         accum_out=ssum)
                    rinv = small.tile([1, 1], F32, tag="rinv")
                    nc.vector.reciprocal(rinv, ssum)
                    nc.vector.tensor_scalar_mul(work_row, p_row, rinv)
                    nc.sync.dma_start(out=attn_out[b:b + 1, :], in_=work_row)

                    if stages < 6:
                        continue
                    # context: DVE multiply-reduce against xt
                    attn_row_d = dram_pool.tile([1, s], F32, tag="attn_row_d")
                    nc.sync.dma_start(out=attn_row_d, in_=work_row)
                    ctx_part = ctxpool.tile([128, NE * NSC], F32, tag="ctx_part")
                    for scix in range(NSC):
                        ssl = slice(scix * sc, (scix + 1) * sc)
                        bc = bcpool.tile([128, sc], F32, tag="bc")
                        nc.gpsimd.dma_start(out=bc,
                                            in_=attn_row_d[0:1, ssl].to_broadcast((128, sc)))
                        for e_t in range(NE):
                            scr = scrpool.tile([128, sc], F32, tag="scr")
                            nc.vector.scalar_tensor_tensor(
                                out=scr, in0=xt[:, e_t, ssl].bitcast(F32),
                                scalar=1.0, in1=bc, op0=OP.mult, op1=OP.mult,
                                accum_out=ctx_part[:, e_t * NSC + scix:
                                                   e_t * NSC + scix + 1])
                    ctx_cols = ctxpool.tile([128, NE], F32, tag="ctx_cols")
                    nc.vector.reduce_sum(
                        out=ctx_cols,
                        in_=ctx_part.rearrange("p (e c) -> p e c", e=NE),
                        axis=mybir.AxisListType.X)
                    nc.sync.dma_start(
                        out=ctx_out[b, :].rearrange("(ek p) -> p ek", p=128),
                        in_=ctx_cols)

    nc.finalize()
    return nc


_NC_CACHE = {}


def _get_nc():
    if "nc" not in _NC_CACHE:
        _NC_CACHE["nc"] = build_nc()
    return _NC_CACHE["nc"]


def kernel(decoder_hidden, encoder_outputs, encoder_mask, coverage,
           W_encoder, W_decoder, W_coverage, v):
    decoder_hidden = np.ascontiguousarray(np.asarray(decoder_hidden, dtype=np.float32))
    encoder_outputs = np.ascontiguousarray(np.asarray(encoder_outputs, dtype=np.float32))
    encoder_mask = np.ascontiguousarray(np.asarray(encoder_mask, dtype=np.int32))
    coverage = np.ascontiguousarray(np.asarray(coverage, dtype=np.float32))
    W_encoder = np.ascontiguousarray(np.asarray(W_encoder, dtype=np.float32))
    W_decoder = np.ascontiguousarray(np.asarray(W_decoder, dtype=np.float32))
    W_coverage = np.ascontiguousarray(np.asarray(W_coverage, dtype=np.float32))
    v = np.ascontiguousarray(np.asarray(v, dtype=np.float32))

    nc = _get_nc()
    core_ids = list(range(N_CORES))
    in_maps = []
    for c in core_ids:
        sl = slice(c * B_LOC, (c + 1) * B_LOC)
        in_maps.append({
            "decoder_hidden": decoder_hidden[sl],
            "encoder_outputs": encoder_outputs[sl],
            "encoder_mask": encoder_mask[sl],
            "coverage": coverage[sl],
            "W_encoder": W_encoder,
            "W_decoder": W_decoder,
            "W_coverage": W_coverage,
            "v": v,
        })
    res = run_bass_kernel_spmd(nc, in_maps, core_ids)
    context = np.concatenate([res.results[c]["context"] for c in core_ids], axis=0)
    attn = np.concatenate([res.results[c]["attn"] for c in core_ids], axis=0)
    return context, attn
